# revision 4
# baseline (speedup 1.0000x reference)
"""Trainium2 Bass kernel for nn_MoEGPT (2-layer MoE GPT forward pass, 8 cores).

Sharding:
  - tokens: 4096 tokens split 512/core (cores 0-3 = batch 0, 4-7 = batch 1)
  - attention: token-sharded; K/V AllGathered within groups of 4 (one batch)
  - MoE: expert-parallel (core c owns expert c for both layers); activations
    AllGathered (8), dense per-expert FFN weighted by top-2 combine weights,
    ReduceScatter sums expert contributions back to token owners
  - lm head: vocab-sharded (4000 cols/core), final activations AllGathered

All compute in fp32. Weights are pre-transposed host-side into the layouts
the PE array wants (contraction dim on partitions).
"""
import numpy as np

P = 128
B, S, D, V = 2, 2048, 768, 32000
L, HN, HD, E, K = 2, 12, 64, 8, 2
FF = 4 * D           # 3072
T = B * S            # 4096
TOKN = 512           # tokens per core
NTS = TOKN // P      # 4 local token subtiles
DC = D // P          # 6
H3 = 3 * D           # 2304
KCH = S // P         # 16 key chunks per batch
FC = FF // P         # 24
TOKCH = 256          # MoE token chunk
NCH = T // TOKCH     # 16
VS = V // 8          # 4000 vocab cols per core
NVC = 8              # vocab col chunks per core
VCW = VS // NVC      # 500
EPS = 1e-5
ET_BUFS = 4

_CACHE = {}


def _build():
    import concourse.bass as bass
    import concourse.mybir as mybir
    import concourse.tile as tile
    from concourse import bacc
    from concourse.masks import make_identity

    f32 = mybir.dt.float32
    i32 = mybir.dt.int32
    AF = mybir.ActivationFunctionType
    OP = mybir.AluOpType

    nc = bacc.Bacc("TRN2", target_bir_lowering=False, debug=False,
                   enable_asserts=True, num_devices=8)

    # ---------------- external inputs (per-core data, same shapes) ----------------
    ids_in = nc.dram_tensor("ids", [P, NTS], i32, kind="ExternalInput")
    pos_in = nc.dram_tensor("pos", [P, NTS, D], f32, kind="ExternalInput")
    emb_in = nc.dram_tensor("tok_emb", [V, D], f32, kind="ExternalInput")
    qkvwT_in = nc.dram_tensor("qkvwT", [L, D, H3], f32, kind="ExternalInput")
    qkvbc_in = nc.dram_tensor("qkvbc", [L, P, H3 // P], f32, kind="ExternalInput")
    qkvbr_in = nc.dram_tensor("qkvbr", [L, 1, H3], f32, kind="ExternalInput")
    projwT_in = nc.dram_tensor("projwT", [L, D, D], f32, kind="ExternalInput")
    projb_in = nc.dram_tensor("projb", [L, 1, D], f32, kind="ExternalInput")
    ln1w_in = nc.dram_tensor("ln1w", [L, D], f32, kind="ExternalInput")
    ln1b_in = nc.dram_tensor("ln1b", [L, D], f32, kind="ExternalInput")
    ln2w_in = nc.dram_tensor("ln2w", [L, D], f32, kind="ExternalInput")
    ln2b_in = nc.dram_tensor("ln2b", [L, D], f32, kind="ExternalInput")
    lnfw_in = nc.dram_tensor("lnfw", [1, D], f32, kind="ExternalInput")
    lnfb_in = nc.dram_tensor("lnfb", [1, D], f32, kind="ExternalInput")
    gatewT_in = nc.dram_tensor("gatewT", [L, D, E], f32, kind="ExternalInput")
    gateb_in = nc.dram_tensor("gateb", [L, 1, E], f32, kind="ExternalInput")
    w1T_in = nc.dram_tensor("w1T", [L, D, FF], f32, kind="ExternalInput")
    b1c_in = nc.dram_tensor("b1c", [L, P, FC], f32, kind="ExternalInput")
    w2T_in = nc.dram_tensor("w2T", [L, FF, D], f32, kind="ExternalInput")
    b2r_in = nc.dram_tensor("b2r", [L, 1, D], f32, kind="ExternalInput")
    ehot_in = nc.dram_tensor("ehot", [1, E], f32, kind="ExternalInput")
    headwT_in = nc.dram_tensor("headwT", [D, VS], f32, kind="ExternalInput")
    headbr_in = nc.dram_tensor("headbr", [1, VS], f32, kind="ExternalInput")

    logits_out = nc.dram_tensor("logits", [TOKN * 8, VS], f32, kind="ExternalOutput")

    groups4 = [[0, 1, 2, 3], [4, 5, 6, 7]]
    groups8 = [list(range(8))]

    with tile.TileContext(nc) as tc:
        with tc.tile_pool(name="dram", bufs=1, space="DRAM") as dram, \
             tc.tile_pool(name="keep", bufs=1) as keep:

            ident = keep.tile([P, P], f32)
            make_identity(nc, ident[:])
            eps_t = keep.tile([P, 1], f32)
            nc.vector.memset(eps_t[:], EPS)
            ones_row = keep.tile([1, P], f32)
            nc.vector.memset(ones_row[:], 1.0)
            ehot_b = keep.tile([P, E], f32)
            nc.sync.dma_start(out=ehot_b[:], in_=ehot_in[:].to_broadcast([P, E]))

            def transpose_128(src_ap, dst_ap, ps_pool):
                tp = ps_pool.tile([P, P], f32, tag="tp", bufs=2)
                nc.tensor.transpose(out=tp[:], in_=src_ap, identity=ident[:])
                nc.vector.tensor_copy(out=dst_ap, in_=tp[:])

            def layer_norm(x_t, w_b, b_b, out_t, pool):
                for ts in range(NTS):
                    xs = x_t[:, ts, :]
                    stats = pool.tile([P, 3, 6], f32, tag="ln_stats", bufs=2)
                    xr = xs.rearrange("p (a b) -> p a b", b=256)
                    for a in range(3):
                        nc.vector.bn_stats(out=stats[:, a, :], in_=xr[:, a, :])
                    mv = pool.tile([P, 2], f32, tag="ln_mv", bufs=2)
                    nc.vector.bn_aggr(out=mv[:], in_=stats[:])
                    sd = pool.tile([P, 1], f32, tag="ln_sd", bufs=2)
                    nc.scalar.activation(out=sd[:], in_=mv[:, 1:2], func=AF.Sqrt,
                                         bias=eps_t[:], scale=1.0)
                    nc.vector.reciprocal(out=sd[:], in_=sd[:])
                    os_ = out_t[:, ts, :]
                    nc.vector.tensor_scalar(out=os_, in0=xs, scalar1=mv[:, 0:1],
                                            scalar2=sd[:], op0=OP.subtract, op1=OP.mult)
                    nc.vector.tensor_tensor(out=os_, in0=os_, in1=w_b[:], op=OP.mult)
                    nc.vector.tensor_tensor(out=os_, in0=os_, in1=b_b[:], op=OP.add)

            x2_dram = [dram.tile([TOKN, D], f32, name=f"x2_dram_{l}") for l in range(L)]
            yp_rs = [dram.tile([TOKN, D], f32, name=f"yp_rs_{l}") for l in range(L)]
            gt_ags = []

            for l in range(L):
                with tc.tile_pool(name=f"pk_{l}", bufs=1) as pk:
                    x = pk.tile([P, NTS, D], f32, name=f"x_{l}")
                    QT = pk.tile([P, DC, TOKN], f32, name=f"QT_{l}")
                    ctxT = pk.tile([P, DC, TOKN], f32, name=f"ctxT_{l}")
                    projwT_sb = pk.tile([P, DC, D], f32, name=f"projwT_{l}")
                    projb_sb = pk.tile([1, D], f32, name=f"projb_{l}")
                    nc.sync.dma_start(out=projwT_sb[:],
                                      in_=projwT_in[l].rearrange("(n p) c -> p n c", p=P))
                    nc.sync.dma_start(out=projb_sb[:], in_=projb_in[l])

                    kt_ag = dram.tile([4 * D, TOKN], f32, name=f"kt_ag_{l}")
                    v_ag = dram.tile([4 * TOKN, HN * (HD + 1)], f32, name=f"v_ag_{l}")

                    # ---- phase A+B: x, LN1, QKV, AllGather K^T / V ----
                    with tc.tile_pool(name=f"pb_{l}", bufs=1) as pb, \
                         tc.tile_pool(name=f"pb_ps_{l}", bufs=2, space="PSUM") as pb_ps:
                        if l == 0:
                            ids_sb = pb.tile([P, NTS], i32, name="ids_sb")
                            nc.sync.dma_start(out=ids_sb[:], in_=ids_in[:])
                            pos_sb = pb.tile([P, NTS, D], f32, name="pos_sb")
                            nc.sync.dma_start(out=pos_sb[:], in_=pos_in[:])
                            for ts in range(NTS):
                                nc.gpsimd.indirect_dma_start(
                                    out=x[:, ts, :], out_offset=None,
                                    in_=emb_in[:],
                                    in_offset=bass.IndirectOffsetOnAxis(
                                        ap=ids_sb[:, ts:ts + 1], axis=0))
                            nc.vector.tensor_tensor(out=x[:], in0=x[:], in1=pos_sb[:],
                                                    op=OP.add)
                        else:
                            yprev = pb.tile([P, NTS, D], f32, name=f"yprev_{l}")
                            xprev = pb.tile([P, NTS, D], f32, name=f"xprev_{l}")
                            nc.sync.dma_start(
                                out=yprev[:],
                                in_=yp_rs[l - 1][:].rearrange("(t p) c -> p t c", p=P))
                            nc.sync.dma_start(
                                out=xprev[:],
                                in_=x2_dram[l - 1][:].rearrange("(t p) c -> p t c", p=P))
                            nc.vector.tensor_tensor(out=x[:], in0=yprev[:], in1=xprev[:],
                                                    op=OP.add)

                        ln1w_b = pb.tile([P, D], f32, name=f"ln1w_{l}")
                        ln1b_b = pb.tile([P, D], f32, name=f"ln1b_{l}")
                        nc.sync.dma_start(out=ln1w_b[:],
                                          in_=ln1w_in[l:l + 1, :].to_broadcast([P, D]))
                        nc.sync.dma_start(out=ln1b_b[:],
                                          in_=ln1b_in[l:l + 1, :].to_broadcast([P, D]))
                        h = pb.tile([P, NTS, D], f32, name=f"h_{l}")
                        layer_norm(x, ln1w_b, ln1b_b, h, pb)

                        hT = pb.tile([P, DC, TOKN], f32, name=f"hT_{l}")
                        for ts in range(NTS):
                            for dc in range(DC):
                                transpose_128(h[:, ts, dc * P:(dc + 1) * P],
                                              hT[:, dc, ts * P:(ts + 1) * P], pb_ps)

                        qkvwT_sb = pb.tile([P, DC, H3], f32, name=f"qkvwT_{l}")
                        nc.sync.dma_start(out=qkvwT_sb[:],
                                          in_=qkvwT_in[l].rearrange("(n p) c -> p n c", p=P))
                        qkvbc_sb = pb.tile([P, H3 // P], f32, name=f"qkvbc_{l}")
                        nc.sync.dma_start(out=qkvbc_sb[:], in_=qkvbc_in[l])
                        qkvbr_sb = pb.tile([1, H3], f32, name=f"qkvbr_{l}")
                        nc.sync.dma_start(out=qkvbr_sb[:], in_=qkvbr_in[l])

                        ktl = pb.tile([P, DC, TOKN], f32, name=f"ktl_{l}")
                        for which, dest in ((0, QT), (1, ktl)):
                            for dd in range(DC):
                                fco = which * D + dd * P
                                ps_qk = pb_ps.tile([P, TOKN], f32, tag="ps_qk", bufs=2)
                                for dc in range(DC):
                                    nc.tensor.matmul(ps_qk[:],
                                                     lhsT=qkvwT_sb[:, dc, fco:fco + P],
                                                     rhs=hT[:, dc, :],
                                                     start=(dc == 0), stop=(dc == DC - 1))
                                bi = which * DC + dd
                                nc.vector.tensor_scalar_add(
                                    out=dest[:, dd, :], in0=ps_qk[:],
                                    scalar1=qkvbc_sb[:, bi:bi + 1])
                        vl = pb.tile([P, NTS, HN * (HD + 1)], f32, name=f"vl_{l}")
                        nc.vector.memset(
                            vl[:].rearrange("p t (h c) -> p t h c", c=HD + 1)[:, :, :, HD:],
                            1.0)
                        for ts in range(NTS):
                            for n0, nw in ((0, 512), (512, 256)):
                                ps_v = pb_ps.tile([P, 512], f32, tag="ps_v", bufs=2)
                                for dc in range(DC):
                                    nc.tensor.matmul(
                                        ps_v[:, :nw],
                                        lhsT=hT[:, dc, ts * P:(ts + 1) * P],
                                        rhs=qkvwT_sb[:, dc, 2 * D + n0:2 * D + n0 + nw],
                                        start=(dc == 0), stop=False)
                                nc.tensor.matmul(
                                    ps_v[:, :nw], lhsT=ones_row[:, :P],
                                    rhs=qkvbr_sb[:, 2 * D + n0:2 * D + n0 + nw],
                                    start=False, stop=True)
                                nh, h0 = nw // HD, n0 // HD
                                nc.vector.tensor_copy(
                                    out=vl[:, ts, :].rearrange("p (h c) -> p h c",
                                                               c=HD + 1)[:, h0:h0 + nh, :HD],
                                    in_=ps_v[:, :nw].rearrange("p (h c) -> p h c", c=HD))

                        kt_in_d = dram.tile([D, TOKN], f32, name=f"kt_in_{l}")
                        v_in_d = dram.tile([TOKN, HN * (HD + 1)], f32, name=f"v_in_{l}")
                        nc.sync.dma_start(out=kt_in_d[:].rearrange("(n p) c -> p n c", p=P),
                                          in_=ktl[:])
                        nc.sync.dma_start(out=v_in_d[:].rearrange("(t p) c -> p t c", p=P),
                                          in_=vl[:])
                        nc.gpsimd.collective_compute(
                            "AllGather", OP.bypass, replica_groups=groups4,
                            ins=[kt_in_d[:].opt()], outs=[kt_ag[:].opt()])
                        nc.gpsimd.collective_compute(
                            "AllGather", OP.bypass, replica_groups=groups4,
                            ins=[v_in_d[:].opt()], outs=[v_ag[:].opt()])

                    # ---- phase C: attention + proj + residual ----
                    with tc.tile_pool(name=f"pc_{l}", bufs=1) as pc, \
                         tc.tile_pool(name=f"pc_ps_{l}", bufs=1, space="PSUM") as pc_ps:
                        kt_sb = pc.tile([P, 4 * DC, TOKN], f32, name=f"kt_sb_{l}")
                        v_sb = pc.tile([P, KCH, HN * (HD + 1)], f32, name=f"v_sb_{l}")
                        nc.sync.dma_start(out=kt_sb[:],
                                          in_=kt_ag[:].rearrange("(n p) c -> p n c", p=P))
                        nc.sync.dma_start(out=v_sb[:],
                                          in_=v_ag[:].rearrange("(n p) c -> p n c", p=P))

                        for hh in range(HN):
                            hp = (hh % 2) * HD
                            hc = hh // 2
                            qt_s = QT[hp:hp + HD, hc, :]
                            ps_ctx = pc_ps.tile([HD + 1, TOKN], f32, tag="ps_ctx", bufs=2)
                            for kc in range(KCH):
                                rb, sc = kc // 4, kc % 4
                                ps_sc = pc_ps.tile([P, TOKN], f32, tag="ps_sc", bufs=2)
                                nc.tensor.matmul(
                                    ps_sc[:],
                                    lhsT=kt_sb[hp:hp + HD, rb * DC + hc, sc * P:(sc + 1) * P],
                                    rhs=qt_s, start=True, stop=True)
                                et = pc.tile([P, TOKN], f32, tag="et", bufs=ET_BUFS)
                                nc.scalar.activation(out=et[:], in_=ps_sc[:], func=AF.Exp,
                                                     scale=1.0 / 8.0)
                                nc.tensor.matmul(
                                    ps_ctx[:],
                                    lhsT=v_sb[:, kc, hh * (HD + 1):(hh + 1) * (HD + 1)],
                                    rhs=et[:],
                                    start=(kc == 0), stop=(kc == KCH - 1))
                            sums = pc.tile([1, TOKN], f32, tag="sums", bufs=2)
                            nc.vector.reciprocal(out=sums[:], in_=ps_ctx[HD:HD + 1, :])
                            ps_bc = pc_ps.tile([HD, TOKN], f32, tag="ps_bc", bufs=1)
                            nc.tensor.matmul(ps_bc[:], lhsT=ones_row[:, :HD], rhs=sums[:],
                                             start=True, stop=True)
                            bc_sb = pc.tile([HD, TOKN], f32, tag="bc_sb", bufs=2)
                            nc.vector.tensor_copy(out=bc_sb[:], in_=ps_bc[:])
                            nc.vector.tensor_tensor(out=ctxT[hp:hp + HD, hc, :],
                                                    in0=ps_ctx[:HD, :], in1=bc_sb[:],
                                                    op=OP.mult)

                        for ts in range(NTS):
                            ps_pr = pc_ps.tile([P, D], f32, tag="ps_pr", bufs=1)
                            for n0, nw in ((0, 512), (512, 256)):
                                for dc in range(DC):
                                    nc.tensor.matmul(ps_pr[:, n0:n0 + nw],
                                                     lhsT=ctxT[:, dc, ts * P:(ts + 1) * P],
                                                     rhs=projwT_sb[:, dc, n0:n0 + nw],
                                                     start=(dc == 0), stop=False)
                                nc.tensor.matmul(ps_pr[:, n0:n0 + nw], lhsT=ones_row[:, :P],
                                                 rhs=projb_sb[:, n0:n0 + nw],
                                                 start=False, stop=True)
                            nc.vector.tensor_tensor(out=x[:, ts, :], in0=ps_pr[:],
                                                    in1=x[:, ts, :], op=OP.add)

                    # ---- phase D: LN2, g^T AllGather, spill x ----
                    with tc.tile_pool(name=f"pd_{l}", bufs=1) as pd, \
                         tc.tile_pool(name=f"pd_ps_{l}", bufs=2, space="PSUM") as pd_ps:
                        ln2w_b = pd.tile([P, D], f32, name=f"ln2w_{l}")
                        ln2b_b = pd.tile([P, D], f32, name=f"ln2b_{l}")
                        nc.sync.dma_start(out=ln2w_b[:],
                                          in_=ln2w_in[l:l + 1, :].to_broadcast([P, D]))
                        nc.sync.dma_start(out=ln2b_b[:],
                                          in_=ln2b_in[l:l + 1, :].to_broadcast([P, D]))
                        g = pd.tile([P, NTS, D], f32, name=f"g_{l}")
                        layer_norm(x, ln2w_b, ln2b_b, g, pd)
                        gtT = pd.tile([P, DC, TOKN], f32, name=f"gtT_{l}")
                        for ts in range(NTS):
                            for dc in range(DC):
                                transpose_128(g[:, ts, dc * P:(dc + 1) * P],
                                              gtT[:, dc, ts * P:(ts + 1) * P], pd_ps)
                        gt_in_d = dram.tile([D, TOKN], f32, name=f"gt_in_{l}")
                        nc.sync.dma_start(out=gt_in_d[:].rearrange("(n p) c -> p n c", p=P),
                                          in_=gtT[:])
                        nc.sync.dma_start(
                            out=x2_dram[l][:].rearrange("(t p) c -> p t c", p=P), in_=x[:])
                        gt_ag = dram.tile([8 * D, TOKN], f32, name=f"gt_ag_{l}",
                                          addr_space="Shared")
                        gt_ags.append(gt_ag)
                        nc.gpsimd.collective_compute(
                            "AllGather", OP.bypass, replica_groups=groups8,
                            ins=[gt_in_d[:].opt()], outs=[gt_ag[:].opt()])

                # ---- phase E: expert-parallel dense MoE ----
                gt_ag = gt_ags[l]
                with tc.tile_pool(name=f"pe_{l}", bufs=1) as pe, \
                     tc.tile_pool(name=f"pe_ps_{l}", bufs=1, space="PSUM") as pe_ps:
                    w1T_sb = pe.tile([P, DC, FF], f32, name=f"w1T_{l}")
                    nc.sync.dma_start(out=w1T_sb[:],
                                      in_=w1T_in[l].rearrange("(n p) c -> p n c", p=P))
                    w2T_sb = pe.tile([P, FC, D], f32, name=f"w2T_{l}")
                    nc.sync.dma_start(out=w2T_sb[:],
                                      in_=w2T_in[l].rearrange("(n p) c -> p n c", p=P))
                    b1c_sb = pe.tile([P, FC], f32, name=f"b1c_{l}")
                    nc.sync.dma_start(out=b1c_sb[:], in_=b1c_in[l])
                    b2r_sb = pe.tile([1, D], f32, name=f"b2r_{l}")
                    nc.sync.dma_start(out=b2r_sb[:], in_=b2r_in[l])
                    gatewT_sb = pe.tile([P, DC, E], f32, name=f"gatewT_{l}")
                    nc.sync.dma_start(out=gatewT_sb[:],
                                      in_=gatewT_in[l].rearrange("(n p) c -> p n c", p=P))
                    gateb_sb = pe.tile([1, E], f32, name=f"gateb_{l}")
                    nc.sync.dma_start(out=gateb_sb[:], in_=gateb_in[l])

                    yp_in_d = dram.tile([T, D], f32, name=f"yp_in_{l}")
                    NSUB = TOKCH // P
                    for cc in range(NCH):
                        gch = pe.tile([P, DC, TOKCH], f32, tag="gch", bufs=2)
                        nc.sync.dma_start(
                            out=gch[:],
                            in_=gt_ag[(cc // 2) * D:(cc // 2 + 1) * D,
                                      (cc % 2) * TOKCH:(cc % 2 + 1) * TOKCH]
                            .rearrange("(n p) c -> p n c", p=P))
                        cwes = []
                        for ts2 in range(NSUB):
                            ps_lg = pe_ps.tile([P, E], f32, tag="ps_lg", bufs=1)
                            for dc in range(DC):
                                nc.tensor.matmul(ps_lg[:],
                                                 lhsT=gch[:, dc, ts2 * P:(ts2 + 1) * P],
                                                 rhs=gatewT_sb[:, dc, :],
                                                 start=(dc == 0), stop=False)
                            nc.tensor.matmul(ps_lg[:], lhsT=ones_row[:, :P],
                                             rhs=gateb_sb[:], start=False, stop=True)
                            lg = pe.tile([P, E], f32, tag="lg", bufs=3)
                            nc.vector.tensor_copy(out=lg[:], in_=ps_lg[:])
                            mx = pe.tile([P, 8], f32, tag="mx", bufs=3)
                            nc.vector.max(out=mx[:], in_=lg[:])
                            ex = pe.tile([P, E], f32, tag="ex", bufs=3)
                            nc.vector.tensor_scalar(out=ex[:], in0=lg[:],
                                                    scalar1=mx[:, 0:1], scalar2=None,
                                                    op0=OP.subtract)
                            nc.scalar.activation(out=ex[:], in_=ex[:], func=AF.Exp)
                            dm = pe.tile([P, 1], f32, tag="dm", bufs=3)
                            nc.vector.tensor_scalar(out=dm[:], in0=mx[:, 1:2],
                                                    scalar1=mx[:, 0:1], scalar2=None,
                                                    op0=OP.subtract)
                            nc.scalar.activation(out=dm[:], in_=dm[:], func=AF.Exp)
                            nc.vector.tensor_scalar_add(out=dm[:], in0=dm[:], scalar1=1.0)
                            nc.vector.reciprocal(out=dm[:], in_=dm[:])
                            msk = pe.tile([P, E], f32, tag="msk", bufs=3)
                            nc.vector.tensor_scalar(out=msk[:], in0=lg[:],
                                                    scalar1=mx[:, 1:2], scalar2=None,
                                                    op0=OP.is_ge)
                            nc.vector.tensor_tensor(out=ex[:], in0=ex[:], in1=msk[:],
                                                    op=OP.mult)
                            nc.vector.tensor_scalar_mul(out=ex[:], in0=ex[:], scalar1=dm[:])
                            nc.vector.tensor_tensor(out=ex[:], in0=ex[:], in1=ehot_b[:],
                                                    op=OP.mult)
                            cwe = pe.tile([P, 1], f32, tag="cwe", bufs=4)
                            nc.vector.reduce_sum(out=cwe[:], in_=ex[:],
                                                 axis=mybir.AxisListType.X)
                            cwes.append(cwe)

                        hT_m = pe.tile([P, FC, TOKCH], f32, tag="hTm", bufs=1)
                        for fc in range(FC):
                            ps_h = pe_ps.tile([P, TOKCH], f32, tag="ps_h", bufs=3)
                            for dc in range(DC):
                                nc.tensor.matmul(ps_h[:],
                                                 lhsT=w1T_sb[:, dc, fc * P:(fc + 1) * P],
                                                 rhs=gch[:, dc, :],
                                                 start=(dc == 0), stop=(dc == DC - 1))
                            nc.scalar.activation(out=hT_m[:, fc, :], in_=ps_h[:],
                                                 func=AF.Gelu_apprx_tanh,
                                                 bias=b1c_sb[:, fc:fc + 1])
                        for ts2 in range(NSUB):
                            ps_y = pe_ps.tile([P, D], f32, tag="ps_y", bufs=2)
                            for n0, nw in ((0, 512), (512, 256)):
                                for fc in range(FC):
                                    nc.tensor.matmul(ps_y[:, n0:n0 + nw],
                                                     lhsT=hT_m[:, fc, ts2 * P:(ts2 + 1) * P],
                                                     rhs=w2T_sb[:, fc, n0:n0 + nw],
                                                     start=(fc == 0), stop=False)
                                nc.tensor.matmul(ps_y[:, n0:n0 + nw], lhsT=ones_row[:, :P],
                                                 rhs=b2r_sb[:, n0:n0 + nw],
                                                 start=False, stop=True)
                            ysc = pe.tile([P, D], f32, tag="ysc", bufs=2)
                            nc.vector.tensor_scalar_mul(out=ysc[:], in0=ps_y[:],
                                                        scalar1=cwes[ts2][:])
                            nc.sync.dma_start(
                                out=yp_in_d[cc * TOKCH + ts2 * P:
                                            cc * TOKCH + (ts2 + 1) * P, :],
                                in_=ysc[:])
                    nc.gpsimd.collective_compute(
                        "ReduceScatter", OP.add, replica_groups=groups8,
                        ins=[yp_in_d[:].opt()], outs=[yp_rs[l][:].opt()])

            # ---- final: LNf, AllGather, lm head ----
            with tc.tile_pool(name="pf", bufs=1) as pf, \
                 tc.tile_pool(name="pf_ps", bufs=2, space="PSUM") as pf_ps:
                x = pf.tile([P, NTS, D], f32, name="x_fin")
                xf = pf.tile([P, NTS, D], f32, name="xf")
                nc.sync.dma_start(out=x[:],
                                  in_=yp_rs[L - 1][:].rearrange("(t p) c -> p t c", p=P))
                nc.sync.dma_start(out=xf[:],
                                  in_=x2_dram[L - 1][:].rearrange("(t p) c -> p t c", p=P))
                nc.vector.tensor_tensor(out=x[:], in0=x[:], in1=xf[:], op=OP.add)
                lnfw_b = pf.tile([P, D], f32, name="lnfw_b")
                lnfb_b = pf.tile([P, D], f32, name="lnfb_b")
                nc.sync.dma_start(out=lnfw_b[:], in_=lnfw_in[:].to_broadcast([P, D]))
                nc.sync.dma_start(out=lnfb_b[:], in_=lnfb_in[:].to_broadcast([P, D]))
                layer_norm(x, lnfw_b, lnfb_b, xf, pf)
                xfT = pf.tile([P, DC, TOKN], f32, name="xfT")
                for ts in range(NTS):
                    for dc in range(DC):
                        transpose_128(xf[:, ts, dc * P:(dc + 1) * P],
                                      xfT[:, dc, ts * P:(ts + 1) * P], pf_ps)
                xf_in_d = dram.tile([D, TOKN], f32, name="xf_in")
                nc.sync.dma_start(out=xf_in_d[:].rearrange("(n p) c -> p n c", p=P),
                                  in_=xfT[:])
                xf_ag = dram.tile([8 * D, TOKN], f32, name="xf_ag", addr_space="Shared")
                nc.gpsimd.collective_compute(
                    "AllGather", OP.bypass, replica_groups=groups8,
                    ins=[xf_in_d[:].opt()], outs=[xf_ag[:].opt()])

                headwT_sb = pf.tile([P, DC, VS], f32, name="headwT_sb")
                nc.sync.dma_start(out=headwT_sb[:],
                                  in_=headwT_in[:].rearrange("(n p) c -> p n c", p=P))
                headbr_sb = pf.tile([1, VS], f32, name="headbr_sb")
                nc.sync.dma_start(out=headbr_sb[:], in_=headbr_in[:])

                for gc in range(8):
                    xch = pf.tile([P, DC, TOKN], f32, tag="xch", bufs=2)
                    nc.sync.dma_start(
                        out=xch[:],
                        in_=xf_ag[gc * D:(gc + 1) * D, :].rearrange("(n p) c -> p n c", p=P))
                    for ts in range(NTS):
                        for vc in range(NVC):
                            ps_lo = pf_ps.tile([P, VCW], f32, tag="ps_lo", bufs=4)
                            for dc in range(DC):
                                nc.tensor.matmul(
                                    ps_lo[:],
                                    lhsT=xch[:, dc, ts * P:(ts + 1) * P],
                                    rhs=headwT_sb[:, dc, vc * VCW:(vc + 1) * VCW],
                                    start=(dc == 0), stop=False)
                            nc.tensor.matmul(ps_lo[:], lhsT=ones_row[:, :P],
                                             rhs=headbr_sb[:, vc * VCW:(vc + 1) * VCW],
                                             start=False, stop=True)
                            lo = pf.tile([P, VCW], f32, tag="lo", bufs=4)
                            if vc % 2 == 0:
                                nc.scalar.copy(out=lo[:], in_=ps_lo[:])
                            else:
                                nc.vector.tensor_copy(out=lo[:], in_=ps_lo[:])
                            nc.sync.dma_start(
                                out=logits_out[gc * TOKN + ts * P:
                                               gc * TOKN + (ts + 1) * P,
                                               vc * VCW:(vc + 1) * VCW],
                                in_=lo[:])

    nc.compile()
    return nc


def _prep_inmaps(inputs):
    ii = {k: np.asarray(v) for k, v in inputs.items()}
    assert int(np.asarray(ii["top_k"])) == K
    assert int(np.asarray(ii["num_heads"])) == HN
    f = np.float32
    ids_flat = ii["input_ids"].astype(np.int32).reshape(T)
    qkvwT = np.ascontiguousarray(ii["qkv_w"].transpose(0, 2, 1)).astype(f)
    qkvbc = np.ascontiguousarray(
        ii["qkv_b"].reshape(L, H3 // P, P).transpose(0, 2, 1)).astype(f)
    qkvbr = ii["qkv_b"].reshape(L, 1, H3).astype(f)
    projwT = np.ascontiguousarray(ii["proj_w"].transpose(0, 2, 1)).astype(f)
    projb = ii["proj_b"].reshape(L, 1, D).astype(f)
    gatewT = np.ascontiguousarray(ii["gate_w"].transpose(0, 2, 1)).astype(f)
    gateb = ii["gate_b"].reshape(L, 1, E).astype(f)
    lnfw = ii["lnf_w"].reshape(1, D).astype(f)
    lnfb = ii["lnf_b"].reshape(1, D).astype(f)
    headb = ii["head_b"].astype(f)
    tok_emb = np.ascontiguousarray(ii["tok_emb"].astype(f))
    pos_emb = np.asarray(ii["pos_emb"]).astype(f)

    in_maps = []
    for c in range(8):
        ids_c = np.ascontiguousarray(ids_flat[c * TOKN:(c + 1) * TOKN].reshape(NTS, P).T)
        s0 = (c % 4) * TOKN
        pos_c = np.ascontiguousarray(
            pos_emb[s0:s0 + TOKN].reshape(NTS, P, D).transpose(1, 0, 2)).astype(f)
        w1T = np.ascontiguousarray(ii["w1"][:, c].transpose(0, 2, 1)).astype(f)
        b1c = np.ascontiguousarray(
            ii["b1"][:, c].reshape(L, FC, P).transpose(0, 2, 1)).astype(f)
        w2T = np.ascontiguousarray(ii["w2"][:, c].transpose(0, 2, 1)).astype(f)
        b2r = ii["b2"][:, c].reshape(L, 1, D).astype(f)
        ehot = np.zeros((1, E), f)
        ehot[0, c] = 1.0
        headwT = np.ascontiguousarray(ii["head_w"][c * VS:(c + 1) * VS].T).astype(f)
        in_maps.append({
            "ids": ids_c, "pos": pos_c, "tok_emb": tok_emb,
            "qkvwT": qkvwT, "qkvbc": qkvbc, "qkvbr": qkvbr,
            "projwT": projwT, "projb": projb,
            "ln1w": ii["ln1_w"].astype(f), "ln1b": ii["ln1_b"].astype(f),
            "ln2w": ii["ln2_w"].astype(f), "ln2b": ii["ln2_b"].astype(f),
            "lnfw": lnfw, "lnfb": lnfb,
            "gatewT": gatewT, "gateb": gateb,
            "w1T": w1T, "b1c": b1c, "w2T": w2T, "b2r": b2r,
            "ehot": ehot,
            "headwT": headwT, "headbr": headb[c * VS:(c + 1) * VS].reshape(1, VS),
        })
    return in_maps


def kernel(**inputs) -> np.ndarray:
    import os
    from concourse.bass_utils import run_bass_kernel_spmd
    if "nc" not in _CACHE:
        _CACHE["nc"] = _build()
    nc = _CACHE["nc"]
    in_maps = _prep_inmaps(inputs)
    trace = bool(int(os.environ.get("KERNEL_TRACE", "0")))
    res = run_bass_kernel_spmd(nc, in_maps, core_ids=list(range(8)), trace=trace)
    _CACHE["last_results"] = res
    full = np.concatenate([res.results[c]["logits"] for c in range(8)], axis=1)
    return full.reshape(B, S, V)
